# revision 14
# baseline (speedup 1.0000x reference)
"""Causal dense self-attention (B=2, T=2048, C=2048, 16 heads, D=128) on 8
Trainium2 NeuronCores.

Sharding: core = b*4 + hg  (b = batch, hg = head-group of 4 heads).
Per core:
  qkv:  x^T arrives pre-transposed (host layout step); qT/kT per head in
        [d, t] layout, v in [t, d] layout; weights streamed once.
        Emitted in 4 rounds (one 512-query slab each) so attention for
        group g starts as soon as slab g's q/k/v are done.
  attn: S^T tiles [t2:128 x t1:<=512] = kT.T @ qT, exact-causal trimmed
        (diagonal tiles only compute the valid query range), exp on ACT,
        diagonal 128-col sub-block masked via gpsimd affine_select.
        PV accumulated on PE; softmax denominators accumulated on DVE
        (tensor_add of P tiles) with a single ones-matmul per (h, group)
        for the partition reduction; normalize on DVE.
  proj: partial c_proj from this core's 4 heads only (contraction split):
        out^T_partial[c_out, t1] = w_proj[own 512 rows].T @ y^T(own),
        written as bf16.  No collectives: the host sums the 4 partial
        outputs per batch (and adds b_proj).
Matmul operands are bf16 (fp32 PSUM accumulate).
"""

import contextlib
import sys

sys.path.insert(0, "/opt/trn_rl_repo")

import ml_dtypes
import numpy as np

import concourse.bacc as bacc
import concourse.mybir as mybir
import concourse.tile as tile
from concourse.bass_utils import run_bass_kernel_spmd

f32 = mybir.dt.float32
bf16 = mybir.dt.bfloat16

T = 2048
C = 2048
N_HEAD_CORE = 4  # heads per core
D = 128
JW = N_HEAD_CORE * D  # 512: per-core slice width of q/k/v
TC = 512  # t1-group width
ATTN_MULT = 1.0 / np.sqrt(D)
N_CORES = 8

_CACHED = {}


def build_nc():
    nc = bacc.Bacc("TRN2", target_bir_lowering=False, debug=False)
    dt = bf16

    xt_d = nc.dram_tensor("xt", [C, T], dt, kind="ExternalInput")
    wq = nc.dram_tensor("wq", [C, JW], dt, kind="ExternalInput")
    wk = nc.dram_tensor("wk", [C, JW], dt, kind="ExternalInput")
    wv = nc.dram_tensor("wv", [C, JW], dt, kind="ExternalInput")
    wp = nc.dram_tensor("wp", [JW, C], dt, kind="ExternalInput")
    bq = nc.dram_tensor("bq", [JW], f32, kind="ExternalInput")
    bk = nc.dram_tensor("bk", [JW], f32, kind="ExternalInput")
    bv = nc.dram_tensor("bv", [JW], dt, kind="ExternalInput")
    ones_d = nc.dram_tensor("ones", [128, 128], dt, kind="ExternalInput")
    outT = nc.dram_tensor("outT", [C, T], dt, kind="ExternalOutput")

    n_cc = C // 128  # 16 contraction chunks
    n_tt = T // 128  # 16 t tiles
    n_rounds = T // TC  # 4

    with tile.TileContext(nc) as tc:
        with contextlib.ExitStack() as ctx:
            const_pool = ctx.enter_context(tc.tile_pool(name="const", bufs=1))
            qkv_pool = ctx.enter_context(tc.tile_pool(name="qkv", bufs=1))
            xt_pool = ctx.enter_context(tc.tile_pool(name="xt", bufs=32))
            wj_pool = ctx.enter_context(tc.tile_pool(name="wj", bufs=1))
            wv_pool = ctx.enter_context(tc.tile_pool(name="wv", bufs=1))
            wp_pool = ctx.enter_context(tc.tile_pool(name="wp", bufs=1))
            p_pool = ctx.enter_context(tc.tile_pool(name="p", bufs=12))
            acc_pool = ctx.enter_context(tc.tile_pool(name="acc", bufs=2))
            accb_pool = ctx.enter_context(tc.tile_pool(name="accb", bufs=2))
            recip_pool = ctx.enter_context(tc.tile_pool(name="recip", bufs=2))
            y_pool = ctx.enter_context(tc.tile_pool(name="y", bufs=12))
            o_pool = ctx.enter_context(tc.tile_pool(name="o", bufs=6))
            chain_psum = ctx.enter_context(
                tc.tile_pool(name="chain_psum", bufs=2, space="PSUM")
            )
            s_psum = ctx.enter_context(
                tc.tile_pool(name="s_psum", bufs=3, space="PSUM")
            )
            y_psum = ctx.enter_context(
                tc.tile_pool(name="y_psum", bufs=2, space="PSUM")
            )
            r_psum = ctx.enter_context(
                tc.tile_pool(name="r_psum", bufs=1, space="PSUM")
            )

            # ---- constants (tiny; gpsimd queue, first so nothing blocks them)
            ones128 = const_pool.tile([128, 128], dt, name="ones128")
            nc.gpsimd.dma_start(out=ones128[:], in_=ones_d.ap())
            ones_row = const_pool.tile([1, 128], dt, name="ones_row")
            nc.gpsimd.dma_start(out=ones_row[:], in_=ones_d.ap()[0:1, :])
            bq_sb = const_pool.tile([128, 4], f32, name="bq_sb")
            bk_sb = const_pool.tile([128, 4], f32, name="bk_sb")
            nc.gpsimd.dma_start(
                out=bq_sb[:], in_=bq.ap().rearrange("(j p) -> p j", p=128)
            )
            nc.gpsimd.dma_start(
                out=bk_sb[:], in_=bk.ap().rearrange("(j p) -> p j", p=128)
            )
            bv_sb = const_pool.tile([1, JW], dt, name="bv_sb")
            nc.gpsimd.dma_start(out=bv_sb[:], in_=bv.ap()[None, :])

            # ---- weights for q/k chains (resident; loaded first) ----
            wj_t = []
            for jj in range(2 * N_HEAD_CORE):
                wj = wj_pool.tile([128, n_cc * 128], dt, name=f"wj_{jj}")
                wj_t.append(wj)

            def load_wj(jj):
                h = jj // 2
                w_dram = wq if jj % 2 == 0 else wk
                nc.scalar.dma_start(
                    out=wj_t[jj][:].rearrange("p (c j) -> p c j", c=n_cc),
                    in_=w_dram.ap()[:, h * 128 : (h + 1) * 128].rearrange(
                        "(c p) j -> p c j", p=128
                    ),
                )

            # q0, k0 first so round-0 chains unblock immediately
            load_wj(0)
            load_wj(1)

            # ---- x^T: 64 tiles [128, TC], tq-major DMA order; slot reuse
            # (bufs=32) throttles slabs 2,3 until slabs 0,1 are consumed.
            xT = [
                [
                    xt_pool.tile([128, TC], dt, tag="xt", name=f"xT_{ci}_{tq}")
                    for ci in range(n_cc)
                ]
                for tq in range(n_rounds)
            ]
            def xt_load(tq, ci):
                eng = nc.sync
                eng.dma_start(
                    out=xT[tq][ci][:],
                    in_=xt_d.ap()[
                        ci * 128 : (ci + 1) * 128, tq * TC : (tq + 1) * TC
                    ],
                )

            for ci in range(n_cc):
                xt_load(0, ci)
            for jj in range(2, 2 * N_HEAD_CORE):
                load_wj(jj)
            for tq in range(1, n_rounds):
                for ci in range(n_cc):
                    xt_load(tq, ci)

            # ---- v weights, then proj weights (needed much later) ----
            wv_t = [
                wv_pool.tile([128, JW], dt, name=f"wv_{ci}") for ci in range(n_cc)
            ]
            for ci in range(n_cc):
                nc.scalar.dma_start(
                    out=wv_t[ci][:], in_=wv.ap()[ci * 128 : (ci + 1) * 128, :]
                )
            wp_t = [
                wp_pool.tile([128, C], dt, name=f"wp_{ci}")
                for ci in range(N_HEAD_CORE)
            ]
            for ci in range(N_HEAD_CORE):
                nc.scalar.dma_start(
                    out=wp_t[ci][:], in_=wp.ap()[ci * 128 : (ci + 1) * 128, :]
                )

            # ---- resident qkv outputs ----
            qT = [
                qkv_pool.tile([128, T], dt, name=f"qT_{h}")
                for h in range(N_HEAD_CORE)
            ]
            kT = [
                qkv_pool.tile([128, T], dt, name=f"kT_{h}")
                for h in range(N_HEAD_CORE)
            ]
            v_sb = [
                qkv_pool.tile([128, JW], dt, name=f"v_{ti}") for ti in range(n_tt)
            ]

            def qk_round(tq):
                for jj in range(2 * N_HEAD_CORE):
                    h = jj // 2
                    is_q = jj % 2 == 0
                    ps = chain_psum.tile([128, TC], f32, tag="mm", name="ps_qk")
                    for ci in range(n_cc):
                        nc.tensor.matmul(
                            ps[:],
                            wj_t[jj][:, ci * 128 : (ci + 1) * 128],
                            xT[tq][ci][:],
                            start=(ci == 0),
                            stop=(ci == n_cc - 1),
                        )
                    dst = qT[h] if is_q else kT[h]
                    bias = bq_sb if is_q else bk_sb
                    if is_q:
                        nc.vector.tensor_scalar_add(
                            dst[:, tq * TC : (tq + 1) * TC],
                            ps[:],
                            bias[:, h : h + 1],
                        )
                    else:
                        nc.scalar.activation(
                            dst[:, tq * TC : (tq + 1) * TC],
                            ps[:],
                            mybir.ActivationFunctionType.Identity,
                            bias=bias[:, h : h + 1],
                        )

            def v_round(tq):
                for ti in range(4 * tq, 4 * tq + 4):
                    ps = chain_psum.tile([128, JW], f32, tag="mm", name="ps_v")
                    tin = ti % 4
                    for ci in range(n_cc):
                        nc.tensor.matmul(
                            ps[:],
                            xT[tq][ci][:, tin * 128 : (tin + 1) * 128],
                            wv_t[ci][:],
                            start=(ci == 0),
                            stop=False,
                        )
                    nc.tensor.matmul(
                        ps[:], ones_row[:], bv_sb[:], start=False, stop=True
                    )
                    nc.vector.tensor_copy(v_sb[ti][:], ps[:])

            y_tiles = {}

            def attn_group(g):
                # diagonal tiles first (full-width r=0 leads so the PSUM
                # start= write covers the whole bank); the last off-diagonal
                # add is fused with the f32->bf16 convert for the ones-matmul.
                order = list(range(4 * g, 4 * g + 4)) + list(range(4 * g))
                for h in range(N_HEAD_CORE):
                    ps_y = y_psum.tile([128, TC], f32, tag="ps_y", name="ps_y")
                    acc = acc_pool.tile([128, TC], f32, tag="acc", name="acc")
                    accb = accb_pool.tile([128, TC], dt, tag="accb", name="accb")
                    for idx, j in enumerate(order):
                        r = j - 4 * g
                        n0 = 128 * r if r > 0 else 0
                        ps_s = s_psum.tile([128, TC], f32, tag="s", name="ps_s")
                        nc.tensor.matmul(
                            ps_s[:, n0:TC],
                            kT[h][:, j * 128 : (j + 1) * 128],
                            qT[h][:, g * TC + n0 : (g + 1) * TC],
                            start=True,
                            stop=True,
                        )
                        pT = p_pool.tile([128, TC], dt, tag="p", name="pT")
                        nc.scalar.activation(
                            pT[:, n0:TC],
                            ps_s[:, n0:TC],
                            mybir.ActivationFunctionType.Exp,
                            scale=float(ATTN_MULT),
                        )
                        if r >= 0:
                            # causal mask on the diagonal 128-col sub-block:
                            # keep iff f' - p >= 0
                            nc.gpsimd.affine_select(
                                out=pT[:, n0 : n0 + 128],
                                in_=pT[:, n0 : n0 + 128],
                                compare_op=mybir.AluOpType.is_ge,
                                fill=0.0,
                                base=0,
                                pattern=[[1, 128]],
                                channel_multiplier=-1,
                            )
                        nc.tensor.matmul(
                            ps_y[:, n0:TC],
                            v_sb[j][:, h * 128 : (h + 1) * 128],
                            pT[:, n0:TC],
                            start=(idx == 0),
                            stop=(idx == len(order) - 1),
                        )
                        if idx == 0:
                            nc.vector.tensor_copy(acc[:], pT[:])
                        elif idx == len(order) - 1 and g > 0:
                            # fused final accumulate + convert to bf16
                            nc.vector.tensor_add(accb[:], acc[:], pT[:])
                        else:
                            nc.vector.tensor_add(
                                acc[:, n0:TC], acc[:, n0:TC], pT[:, n0:TC]
                            )
                    if g == 0:
                        nc.vector.tensor_copy(accb[:], acc[:])
                    ps_r = r_psum.tile([128, TC], f32, tag="ps_r", name="ps_r")
                    nc.tensor.matmul(
                        ps_r[:], ones128[:], accb[:], start=True, stop=True
                    )
                    recip = recip_pool.tile(
                        [128, TC], f32, tag="recip", name="recip"
                    )
                    nc.vector.reciprocal_approx_fast(out=recip[:], in_=ps_r[:])
                    y_sb = y_pool.tile([128, TC], dt, tag="y", name="y_sb")
                    nc.vector.tensor_mul(y_sb[:], ps_y[:], recip[:])
                    y_tiles[(g, h)] = y_sb

            def proj_group(g):
                for co in range(C // 128):
                    ps_o = chain_psum.tile([128, TC], f32, tag="mm", name="ps_o")
                    for ci in range(N_HEAD_CORE):
                        nc.tensor.matmul(
                            ps_o[:],
                            wp_t[ci][:, co * 128 : (co + 1) * 128],
                            y_tiles[(g, ci)][:],
                            start=(ci == 0),
                            stop=(ci == N_HEAD_CORE - 1),
                        )
                    o_sb = o_pool.tile([128, TC], dt, tag="o", name="o_sb")
                    if co % 2 == 0:
                        nc.scalar.copy(o_sb[:], ps_o[:])
                    else:
                        nc.vector.tensor_copy(o_sb[:], ps_o[:])
                    nc.sync.dma_start(
                        out=outT.ap()[
                            co * 128 : (co + 1) * 128, g * TC : (g + 1) * TC
                        ],
                        in_=o_sb[:],
                    )

            # emission order = scheduler priority
            for g in range(n_rounds):
                qk_round(g)
                v_round(g)
                attn_group(g)
                if g == 3:
                    proj_group(0)
            proj_group(1)
            proj_group(2)
            proj_group(3)

    nc.compile()
    return nc


def kernel(x, w_qkv, b_qkv, w_proj, b_proj, _trace=False):
    x = np.ascontiguousarray(np.asarray(x, dtype=np.float32))
    w_qkv = np.ascontiguousarray(np.asarray(w_qkv, dtype=np.float32))
    b_qkv = np.ascontiguousarray(np.asarray(b_qkv, dtype=np.float32))
    w_proj = np.ascontiguousarray(np.asarray(w_proj, dtype=np.float32))
    b_proj = np.ascontiguousarray(np.asarray(b_proj, dtype=np.float32))
    B = x.shape[0]

    if "nc" not in _CACHED:
        _CACHED["nc"] = build_nc()
    nc = _CACHED["nc"]

    np_dt = ml_dtypes.bfloat16

    def cvt(a):
        return np.ascontiguousarray(a.astype(np_dt))

    in_maps = []
    for core in range(N_CORES):
        b, hg = divmod(core, 4)
        s = slice(hg * JW, (hg + 1) * JW)
        in_maps.append(
            {
                "xt": cvt(np.ascontiguousarray(x[b].T)),
                "wq": cvt(w_qkv[:, 0:C][:, s]),
                "wk": cvt(w_qkv[:, C : 2 * C][:, s]),
                "wv": cvt(w_qkv[:, 2 * C : 3 * C][:, s]),
                "wp": cvt(w_proj[s, :]),
                "bq": np.ascontiguousarray(b_qkv[0:C][s]),
                "bk": np.ascontiguousarray(b_qkv[C : 2 * C][s]),
                "bv": cvt(b_qkv[2 * C : 3 * C][s]),
                "ones": np.ones((128, 128), dtype=np_dt),
            }
        )

    res = run_bass_kernel_spmd(nc, in_maps, list(range(N_CORES)), trace=_trace)
    _CACHED["last_result"] = res

    out = np.zeros((B, T, C), dtype=np.float32)
    for core in range(N_CORES):
        b, hg = divmod(core, 4)
        out[b] += res.results[core]["outT"].T.astype(np.float32)
    out += b_proj
    return np.ascontiguousarray(out)


# revision 17
# speedup vs baseline: 1.0331x; 1.0331x over previous
"""Causal dense self-attention (B=2, T=2048, C=2048, 16 heads, D=128) on 8
Trainium2 NeuronCores.

Sharding: core = b*4 + hg  (b = batch, hg = head-group of 4 heads).
Per core:
  qkv:  x^T arrives pre-transposed (host layout step); qT/kT per head in
        [d, t] layout, v in [t, d] layout; weights streamed once.
        Emitted in 4 rounds (one 512-query slab each) so attention for
        group g starts as soon as slab g's q/k/v are done.
  attn: S^T tiles [t2:128 x t1:<=512] = kT.T @ qT, exact-causal trimmed
        (diagonal tiles only compute the valid query range), exp on ACT,
        diagonal 128-col sub-block masked via gpsimd affine_select.
        PV accumulated on PE; softmax denominators accumulated on DVE
        (tensor_add of P tiles) with a single ones-matmul per (h, group)
        for the partition reduction; normalize on DVE.
  proj: partial c_proj from this core's 4 heads only (contraction split):
        out^T_partial[c_out, t1] = w_proj[own 512 rows].T @ y^T(own),
        written as bf16.  No collectives: the host sums the 4 partial
        outputs per batch (and adds b_proj).
Matmul operands are bf16 (fp32 PSUM accumulate).
"""

import contextlib
import sys

sys.path.insert(0, "/opt/trn_rl_repo")

import ml_dtypes
import numpy as np

import concourse.bacc as bacc
import concourse.mybir as mybir
import concourse.tile as tile
from concourse.bass_utils import run_bass_kernel_spmd

f32 = mybir.dt.float32
bf16 = mybir.dt.bfloat16

T = 2048
C = 2048
N_HEAD_CORE = 4  # heads per core
D = 128
JW = N_HEAD_CORE * D  # 512: per-core slice width of q/k/v
TC = 512  # t1-group width
ATTN_MULT = 1.0 / np.sqrt(D)
N_CORES = 8

_CACHED = {}


def build_nc():
    nc = bacc.Bacc("TRN2", target_bir_lowering=False, debug=False)
    dt = bf16

    xt_d = nc.dram_tensor("xt", [C, T], dt, kind="ExternalInput")
    wq = nc.dram_tensor("wq", [C, JW], dt, kind="ExternalInput")
    wk = nc.dram_tensor("wk", [C, JW], dt, kind="ExternalInput")
    wv = nc.dram_tensor("wv", [C, JW], dt, kind="ExternalInput")
    wp = nc.dram_tensor("wp", [JW, C], dt, kind="ExternalInput")
    bq = nc.dram_tensor("bq", [JW], f32, kind="ExternalInput")
    bk = nc.dram_tensor("bk", [JW], f32, kind="ExternalInput")
    bv = nc.dram_tensor("bv", [JW], dt, kind="ExternalInput")
    ones_d = nc.dram_tensor("ones", [128, 128], dt, kind="ExternalInput")
    outT = nc.dram_tensor("outT", [C, T], dt, kind="ExternalOutput")

    n_cc = C // 128  # 16 contraction chunks
    n_tt = T // 128  # 16 t tiles
    n_rounds = T // TC  # 4

    with tile.TileContext(nc) as tc:
        with contextlib.ExitStack() as ctx:
            const_pool = ctx.enter_context(tc.tile_pool(name="const", bufs=1))
            qkv_pool = ctx.enter_context(tc.tile_pool(name="qkv", bufs=1))
            xt_pool = ctx.enter_context(tc.tile_pool(name="xt", bufs=32))
            wj_pool = ctx.enter_context(tc.tile_pool(name="wj", bufs=1))
            wv_pool = ctx.enter_context(tc.tile_pool(name="wv", bufs=1))
            wp_pool = ctx.enter_context(tc.tile_pool(name="wp", bufs=1))
            p_pool = ctx.enter_context(tc.tile_pool(name="p", bufs=12))
            acc_pool = ctx.enter_context(tc.tile_pool(name="acc", bufs=2))
            accb_pool = ctx.enter_context(tc.tile_pool(name="accb", bufs=2))
            recip_pool = ctx.enter_context(tc.tile_pool(name="recip", bufs=2))
            y_pool = ctx.enter_context(tc.tile_pool(name="y", bufs=12))
            o_pool = ctx.enter_context(tc.tile_pool(name="o", bufs=6))
            chain_psum = ctx.enter_context(
                tc.tile_pool(name="chain_psum", bufs=2, space="PSUM")
            )
            s_psum = ctx.enter_context(
                tc.tile_pool(name="s_psum", bufs=3, space="PSUM")
            )
            y_psum = ctx.enter_context(
                tc.tile_pool(name="y_psum", bufs=2, space="PSUM")
            )
            r_psum = ctx.enter_context(
                tc.tile_pool(name="r_psum", bufs=1, space="PSUM")
            )

            # ---- constants (tiny; gpsimd queue, first so nothing blocks them)
            ones128 = const_pool.tile([128, 128], dt, name="ones128")
            nc.gpsimd.dma_start(out=ones128[:], in_=ones_d.ap())
            ones_row = const_pool.tile([1, 128], dt, name="ones_row")
            nc.gpsimd.dma_start(out=ones_row[:], in_=ones_d.ap()[0:1, :])
            bq_sb = const_pool.tile([128, 4], f32, name="bq_sb")
            bk_sb = const_pool.tile([128, 4], f32, name="bk_sb")
            nc.gpsimd.dma_start(
                out=bq_sb[:], in_=bq.ap().rearrange("(j p) -> p j", p=128)
            )
            nc.gpsimd.dma_start(
                out=bk_sb[:], in_=bk.ap().rearrange("(j p) -> p j", p=128)
            )
            bv_sb = const_pool.tile([1, JW], dt, name="bv_sb")
            nc.gpsimd.dma_start(out=bv_sb[:], in_=bv.ap()[None, :])

            # ---- v / proj weights on the gpsimd queue: these posts never
            # wait on slots, so they cannot block the affine_selects.
            wv_t = [
                wv_pool.tile([128, JW], dt, name=f"wv_{ci}") for ci in range(n_cc)
            ]
            for ci in range(n_cc):
                nc.gpsimd.dma_start(
                    out=wv_t[ci][:], in_=wv.ap()[ci * 128 : (ci + 1) * 128, :]
                )
            wp_t = [
                wp_pool.tile([128, C], dt, name=f"wp_{ci}")
                for ci in range(N_HEAD_CORE)
            ]
            for ci in range(N_HEAD_CORE):
                nc.gpsimd.dma_start(
                    out=wp_t[ci][:], in_=wp.ap()[ci * 128 : (ci + 1) * 128, :]
                )

            # ---- weights for q/k chains (resident; loaded first) ----
            wj_t = []
            for jj in range(2 * N_HEAD_CORE):
                wj = wj_pool.tile([128, n_cc * 128], dt, name=f"wj_{jj}")
                wj_t.append(wj)

            def load_wj(jj):
                h = jj // 2
                w_dram = wq if jj % 2 == 0 else wk
                nc.scalar.dma_start(
                    out=wj_t[jj][:].rearrange("p (c j) -> p c j", c=n_cc),
                    in_=w_dram.ap()[:, h * 128 : (h + 1) * 128].rearrange(
                        "(c p) j -> p c j", p=128
                    ),
                )

            # q0, k0 first so round-0 chains unblock immediately
            load_wj(0)
            load_wj(1)

            # ---- x^T: 64 tiles [128, TC], tq-major DMA order; slot reuse
            # (bufs=32) throttles slabs 2,3 until slabs 0,1 are consumed.
            xT = [
                [
                    xt_pool.tile([128, TC], dt, tag="xt", name=f"xT_{ci}_{tq}")
                    for ci in range(n_cc)
                ]
                for tq in range(n_rounds)
            ]
            def xt_load(tq, ci):
                eng = nc.sync
                eng.dma_start(
                    out=xT[tq][ci][:],
                    in_=xt_d.ap()[
                        ci * 128 : (ci + 1) * 128, tq * TC : (tq + 1) * TC
                    ],
                )

            for ci in range(n_cc):
                xt_load(0, ci)
            for jj in range(2, 2 * N_HEAD_CORE):
                load_wj(jj)
            for tq in range(1, n_rounds):
                for ci in range(n_cc):
                    xt_load(tq, ci)

            # ---- resident qkv outputs ----
            qT = [
                qkv_pool.tile([128, T], dt, name=f"qT_{h}")
                for h in range(N_HEAD_CORE)
            ]
            kT = [
                qkv_pool.tile([128, T], dt, name=f"kT_{h}")
                for h in range(N_HEAD_CORE)
            ]
            v_sb = [
                qkv_pool.tile([128, JW], dt, name=f"v_{ti}") for ti in range(n_tt)
            ]

            def qk_round(tq):
                for jj in range(2 * N_HEAD_CORE):
                    h = jj // 2
                    is_q = jj % 2 == 0
                    ps = chain_psum.tile([128, TC], f32, tag="mm", name="ps_qk")
                    for ci in range(n_cc):
                        nc.tensor.matmul(
                            ps[:],
                            wj_t[jj][:, ci * 128 : (ci + 1) * 128],
                            xT[tq][ci][:],
                            start=(ci == 0),
                            stop=(ci == n_cc - 1),
                        )
                    dst = qT[h] if is_q else kT[h]
                    bias = bq_sb if is_q else bk_sb
                    nc.vector.tensor_scalar_add(
                        dst[:, tq * TC : (tq + 1) * TC],
                        ps[:],
                        bias[:, h : h + 1],
                    )

            def v_round(tq):
                for ti in range(4 * tq, 4 * tq + 4):
                    ps = chain_psum.tile([128, JW], f32, tag="mm", name="ps_v")
                    tin = ti % 4
                    for ci in range(n_cc):
                        nc.tensor.matmul(
                            ps[:],
                            xT[tq][ci][:, tin * 128 : (tin + 1) * 128],
                            wv_t[ci][:],
                            start=(ci == 0),
                            stop=False,
                        )
                    nc.tensor.matmul(
                        ps[:], ones_row[:], bv_sb[:], start=False, stop=True
                    )
                    nc.vector.tensor_copy(v_sb[ti][:], ps[:])

            y_tiles = {}

            def attn_group(g):
                # diagonal tiles first (full-width r=0 leads so the PSUM
                # start= write covers the whole bank); the last off-diagonal
                # add is fused with the f32->bf16 convert for the ones-matmul.
                order = list(range(4 * g, 4 * g + 4)) + list(range(4 * g))
                for h in range(N_HEAD_CORE):
                    ps_y = y_psum.tile([128, TC], f32, tag="ps_y", name="ps_y")
                    acc = acc_pool.tile([128, TC], f32, tag="acc", name="acc")
                    accb = accb_pool.tile([128, TC], dt, tag="accb", name="accb")
                    for idx, j in enumerate(order):
                        r = j - 4 * g
                        n0 = 128 * r if r > 0 else 0
                        ps_s = s_psum.tile([128, TC], f32, tag="s", name="ps_s")
                        nc.tensor.matmul(
                            ps_s[:, n0:TC],
                            kT[h][:, j * 128 : (j + 1) * 128],
                            qT[h][:, g * TC + n0 : (g + 1) * TC],
                            start=True,
                            stop=True,
                        )
                        pT = p_pool.tile([128, TC], dt, tag="p", name="pT")
                        nc.scalar.activation(
                            pT[:, n0:TC],
                            ps_s[:, n0:TC],
                            mybir.ActivationFunctionType.Exp,
                            scale=float(ATTN_MULT),
                        )
                        if r >= 0:
                            # causal mask on the diagonal 128-col sub-block:
                            # keep iff f' - p >= 0
                            nc.gpsimd.affine_select(
                                out=pT[:, n0 : n0 + 128],
                                in_=pT[:, n0 : n0 + 128],
                                compare_op=mybir.AluOpType.is_ge,
                                fill=0.0,
                                base=0,
                                pattern=[[1, 128]],
                                channel_multiplier=-1,
                            )
                        nc.tensor.matmul(
                            ps_y[:, n0:TC],
                            v_sb[j][:, h * 128 : (h + 1) * 128],
                            pT[:, n0:TC],
                            start=(idx == 0),
                            stop=(idx == len(order) - 1),
                        )
                        if idx == 0:
                            nc.vector.tensor_copy(acc[:], pT[:])
                        elif idx == len(order) - 1 and g > 0:
                            # fused final accumulate + convert to bf16
                            nc.vector.tensor_add(accb[:], acc[:], pT[:])
                        else:
                            nc.vector.tensor_add(
                                acc[:, n0:TC], acc[:, n0:TC], pT[:, n0:TC]
                            )
                    if g == 0:
                        nc.vector.tensor_copy(accb[:], acc[:])
                    ps_r = r_psum.tile([128, TC], f32, tag="ps_r", name="ps_r")
                    nc.tensor.matmul(
                        ps_r[:], ones128[:], accb[:], start=True, stop=True
                    )
                    recip = recip_pool.tile(
                        [128, TC], f32, tag="recip", name="recip"
                    )
                    nc.vector.reciprocal_approx_fast(out=recip[:], in_=ps_r[:])
                    y_sb = y_pool.tile([128, TC], dt, tag="y", name="y_sb")
                    nc.vector.tensor_mul(y_sb[:], ps_y[:], recip[:])
                    y_tiles[(g, h)] = y_sb

            def proj_group(g):
                for co in range(C // 128):
                    ps_o = chain_psum.tile([128, TC], f32, tag="mm", name="ps_o")
                    for ci in range(N_HEAD_CORE):
                        nc.tensor.matmul(
                            ps_o[:],
                            wp_t[ci][:, co * 128 : (co + 1) * 128],
                            y_tiles[(g, ci)][:],
                            start=(ci == 0),
                            stop=(ci == N_HEAD_CORE - 1),
                        )
                    o_sb = o_pool.tile([128, TC], dt, tag="o", name="o_sb")
                    if co % 2 == 0:
                        nc.scalar.copy(o_sb[:], ps_o[:])
                    else:
                        nc.vector.tensor_copy(o_sb[:], ps_o[:])
                    nc.sync.dma_start(
                        out=outT.ap()[
                            co * 128 : (co + 1) * 128, g * TC : (g + 1) * TC
                        ],
                        in_=o_sb[:],
                    )

            # emission order = scheduler priority
            for g in range(n_rounds):
                qk_round(g)
                v_round(g)
                attn_group(g)
                if g == 3:
                    proj_group(0)
            proj_group(1)
            proj_group(2)
            proj_group(3)

    nc.compile()
    return nc


def kernel(x, w_qkv, b_qkv, w_proj, b_proj, _trace=False):
    x = np.ascontiguousarray(np.asarray(x, dtype=np.float32))
    w_qkv = np.ascontiguousarray(np.asarray(w_qkv, dtype=np.float32))
    b_qkv = np.ascontiguousarray(np.asarray(b_qkv, dtype=np.float32))
    w_proj = np.ascontiguousarray(np.asarray(w_proj, dtype=np.float32))
    b_proj = np.ascontiguousarray(np.asarray(b_proj, dtype=np.float32))
    B = x.shape[0]

    if "nc" not in _CACHED:
        _CACHED["nc"] = build_nc()
    nc = _CACHED["nc"]

    np_dt = ml_dtypes.bfloat16

    def cvt(a):
        return np.ascontiguousarray(a.astype(np_dt))

    in_maps = []
    for core in range(N_CORES):
        b, hg = divmod(core, 4)
        s = slice(hg * JW, (hg + 1) * JW)
        in_maps.append(
            {
                "xt": cvt(np.ascontiguousarray(x[b].T)),
                "wq": cvt(w_qkv[:, 0:C][:, s]),
                "wk": cvt(w_qkv[:, C : 2 * C][:, s]),
                "wv": cvt(w_qkv[:, 2 * C : 3 * C][:, s]),
                "wp": cvt(w_proj[s, :]),
                "bq": np.ascontiguousarray(b_qkv[0:C][s]),
                "bk": np.ascontiguousarray(b_qkv[C : 2 * C][s]),
                "bv": cvt(b_qkv[2 * C : 3 * C][s]),
                "ones": np.ones((128, 128), dtype=np_dt),
            }
        )

    res = run_bass_kernel_spmd(nc, in_maps, list(range(N_CORES)), trace=_trace)
    _CACHED["last_result"] = res

    out = np.zeros((B, T, C), dtype=np.float32)
    for core in range(N_CORES):
        b, hg = divmod(core, 4)
        out[b] += res.results[core]["outT"].T.astype(np.float32)
    out += b_proj
    return np.ascontiguousarray(out)


# revision 19
# speedup vs baseline: 1.0342x; 1.0010x over previous
"""Causal dense self-attention (B=2, T=2048, C=2048, 16 heads, D=128) on 8
Trainium2 NeuronCores.

Sharding: core = b*4 + hg  (b = batch, hg = head-group of 4 heads).
Per core:
  qkv:  x^T arrives pre-transposed (host layout step); qT/kT per head in
        [d, t] layout, v in [t, d] layout; weights streamed once.
        Emitted in 4 rounds (one 512-query slab each) so attention for
        group g starts as soon as slab g's q/k/v are done.
  attn: S^T tiles [t2:128 x t1:<=512] = kT.T @ qT, exact-causal trimmed
        (diagonal tiles only compute the valid query range), exp on ACT,
        diagonal 128-col sub-block masked via gpsimd affine_select.
        PV accumulated on PE; softmax denominators accumulated on DVE
        (tensor_add of P tiles) with a single ones-matmul per (h, group)
        for the partition reduction; normalize on DVE.
  proj: partial c_proj from this core's 4 heads only (contraction split):
        out^T_partial[c_out, t1] = w_proj[own 512 rows].T @ y^T(own),
        written as bf16.  No collectives: the host sums the 4 partial
        outputs per batch (and adds b_proj).
Matmul operands are bf16 (fp32 PSUM accumulate).
"""

import contextlib
import sys

sys.path.insert(0, "/opt/trn_rl_repo")

import ml_dtypes
import numpy as np

import concourse.bacc as bacc
import concourse.mybir as mybir
import concourse.tile as tile
from concourse.bass_utils import run_bass_kernel_spmd

f32 = mybir.dt.float32
bf16 = mybir.dt.bfloat16

T = 2048
C = 2048
N_HEAD_CORE = 4  # heads per core
D = 128
JW = N_HEAD_CORE * D  # 512: per-core slice width of q/k/v
TC = 512  # t1-group width
ATTN_MULT = 1.0 / np.sqrt(D)
N_CORES = 8

_CACHED = {}


def build_nc():
    nc = bacc.Bacc("TRN2", target_bir_lowering=False, debug=False)
    dt = bf16

    xt_d = nc.dram_tensor("xt", [C, T], dt, kind="ExternalInput")
    wq = nc.dram_tensor("wq", [C, JW], dt, kind="ExternalInput")
    wk = nc.dram_tensor("wk", [C, JW], dt, kind="ExternalInput")
    wv = nc.dram_tensor("wv", [C, JW], dt, kind="ExternalInput")
    wp = nc.dram_tensor("wp", [JW, C], dt, kind="ExternalInput")
    bq = nc.dram_tensor("bq", [JW], f32, kind="ExternalInput")
    bk = nc.dram_tensor("bk", [JW], f32, kind="ExternalInput")
    bv = nc.dram_tensor("bv", [JW], dt, kind="ExternalInput")
    ones_d = nc.dram_tensor("ones", [128, 128], dt, kind="ExternalInput")
    outT = nc.dram_tensor("outT", [C, T], dt, kind="ExternalOutput")

    n_cc = C // 128  # 16 contraction chunks
    n_tt = T // 128  # 16 t tiles
    n_rounds = T // TC  # 4

    with tile.TileContext(nc) as tc:
        with contextlib.ExitStack() as ctx:
            const_pool = ctx.enter_context(tc.tile_pool(name="const", bufs=1))
            qkv_pool = ctx.enter_context(tc.tile_pool(name="qkv", bufs=1))
            xt_pool = ctx.enter_context(tc.tile_pool(name="xt", bufs=2))
            wj_pool = ctx.enter_context(tc.tile_pool(name="wj", bufs=1))
            wv_pool = ctx.enter_context(tc.tile_pool(name="wv", bufs=1))
            wp_pool = ctx.enter_context(tc.tile_pool(name="wp", bufs=1))
            p_pool = ctx.enter_context(tc.tile_pool(name="p", bufs=12))
            acc_pool = ctx.enter_context(tc.tile_pool(name="acc", bufs=2))
            accb_pool = ctx.enter_context(tc.tile_pool(name="accb", bufs=2))
            recip_pool = ctx.enter_context(tc.tile_pool(name="recip", bufs=2))
            y_pool = ctx.enter_context(tc.tile_pool(name="y", bufs=12))
            o_pool = ctx.enter_context(tc.tile_pool(name="o", bufs=6))
            chain_psum = ctx.enter_context(
                tc.tile_pool(name="chain_psum", bufs=2, space="PSUM")
            )
            s_psum = ctx.enter_context(
                tc.tile_pool(name="s_psum", bufs=3, space="PSUM")
            )
            y_psum = ctx.enter_context(
                tc.tile_pool(name="y_psum", bufs=2, space="PSUM")
            )
            r_psum = ctx.enter_context(
                tc.tile_pool(name="r_psum", bufs=1, space="PSUM")
            )

            # ---- constants (tiny; gpsimd queue, first so nothing blocks them)
            ones128 = const_pool.tile([128, 128], dt, name="ones128")
            nc.gpsimd.dma_start(out=ones128[:], in_=ones_d.ap())
            ones_row = const_pool.tile([1, 128], dt, name="ones_row")
            nc.gpsimd.dma_start(out=ones_row[:], in_=ones_d.ap()[0:1, :])
            bq_sb = const_pool.tile([128, 4], f32, name="bq_sb")
            bk_sb = const_pool.tile([128, 4], f32, name="bk_sb")
            nc.gpsimd.dma_start(
                out=bq_sb[:], in_=bq.ap().rearrange("(j p) -> p j", p=128)
            )
            nc.gpsimd.dma_start(
                out=bk_sb[:], in_=bk.ap().rearrange("(j p) -> p j", p=128)
            )
            bv_sb = const_pool.tile([1, JW], dt, name="bv_sb")
            nc.gpsimd.dma_start(out=bv_sb[:], in_=bv.ap()[None, :])

            # ---- v / proj weights on the gpsimd queue: these posts never
            # wait on slots, so they cannot block the affine_selects.
            # wv as ONE 2MB post (big transfers reach ~340GB/s; small ones
            # are descriptor-dominated).
            wv_sb = wv_pool.tile([128, n_cc * JW], dt, name="wv_sb")
            nc.gpsimd.dma_start(
                out=wv_sb[:].rearrange("p (c j) -> p c j", c=n_cc),
                in_=wv.ap().rearrange("(c p) j -> p c j", p=128),
            )
            wp_t = [
                wp_pool.tile([128, C], dt, name=f"wp_{ci}")
                for ci in range(N_HEAD_CORE)
            ]
            for ci in range(N_HEAD_CORE):
                nc.gpsimd.dma_start(
                    out=wp_t[ci][:], in_=wp.ap()[ci * 128 : (ci + 1) * 128, :]
                )

            # ---- weights for q/k chains (resident; loaded first) ----
            wj_t = []
            for jj in range(2 * N_HEAD_CORE):
                wj = wj_pool.tile([128, n_cc * 128], dt, name=f"wj_{jj}")
                wj_t.append(wj)

            def load_wj(jj):
                h = jj // 2
                w_dram = wq if jj % 2 == 0 else wk
                nc.scalar.dma_start(
                    out=wj_t[jj][:].rearrange("p (c j) -> p c j", c=n_cc),
                    in_=w_dram.ap()[:, h * 128 : (h + 1) * 128].rearrange(
                        "(c p) j -> p c j", p=128
                    ),
                )

            # q0, k0 first so round-0 chains unblock immediately
            load_wj(0)
            load_wj(1)

            # ---- x^T: one [128, 16*TC] slab tile per round, filled by 4
            # posts of 512KB each (ci-groups of 4); bufs=2 slot reuse
            # throttles slabs 2,3 until slabs 0,1 are consumed.
            xS = [
                xt_pool.tile([128, n_cc * TC], dt, tag="xt", name=f"xS_{tq}")
                for tq in range(n_rounds)
            ]

            def xt_load(tq):
                for k in range(4):
                    nc.sync.dma_start(
                        out=xS[tq][:, k * 4 * TC : (k + 1) * 4 * TC].rearrange(
                            "p (c t) -> p c t", c=4
                        ),
                        in_=xt_d.ap()[
                            k * 512 : (k + 1) * 512, tq * TC : (tq + 1) * TC
                        ].rearrange("(c p) t -> p c t", p=128),
                    )

            def xt_ap(tq, ci):
                return xS[tq][:, ci * TC : (ci + 1) * TC]

            xt_load(0)
            for jj in range(2, 2 * N_HEAD_CORE):
                load_wj(jj)
            for tq in range(1, n_rounds):
                xt_load(tq)

            # ---- resident qkv outputs ----
            qT = [
                qkv_pool.tile([128, T], dt, name=f"qT_{h}")
                for h in range(N_HEAD_CORE)
            ]
            kT = [
                qkv_pool.tile([128, T], dt, name=f"kT_{h}")
                for h in range(N_HEAD_CORE)
            ]
            v_sb = [
                qkv_pool.tile([128, JW], dt, name=f"v_{ti}") for ti in range(n_tt)
            ]

            def qk_round(tq):
                for jj in range(2 * N_HEAD_CORE):
                    h = jj // 2
                    is_q = jj % 2 == 0
                    ps = chain_psum.tile([128, TC], f32, tag="mm", name="ps_qk")
                    for ci in range(n_cc):
                        nc.tensor.matmul(
                            ps[:],
                            wj_t[jj][:, ci * 128 : (ci + 1) * 128],
                            xt_ap(tq, ci),
                            start=(ci == 0),
                            stop=(ci == n_cc - 1),
                        )
                    dst = qT[h] if is_q else kT[h]
                    bias = bq_sb if is_q else bk_sb
                    nc.vector.tensor_scalar_add(
                        dst[:, tq * TC : (tq + 1) * TC],
                        ps[:],
                        bias[:, h : h + 1],
                    )

            def v_round(tq):
                for ti in range(4 * tq, 4 * tq + 4):
                    ps = chain_psum.tile([128, JW], f32, tag="mm", name="ps_v")
                    tin = ti % 4
                    for ci in range(n_cc):
                        nc.tensor.matmul(
                            ps[:],
                            xt_ap(tq, ci)[:, tin * 128 : (tin + 1) * 128],
                            wv_sb[:, ci * JW : (ci + 1) * JW],
                            start=(ci == 0),
                            stop=False,
                        )
                    nc.tensor.matmul(
                        ps[:], ones_row[:], bv_sb[:], start=False, stop=True
                    )
                    nc.vector.tensor_copy(v_sb[ti][:], ps[:])

            y_tiles = {}

            def attn_group(g):
                # diagonal tiles first (full-width r=0 leads so the PSUM
                # start= write covers the whole bank); the last off-diagonal
                # add is fused with the f32->bf16 convert for the ones-matmul.
                order = list(range(4 * g, 4 * g + 4)) + list(range(4 * g))
                for h in range(N_HEAD_CORE):
                    ps_y = y_psum.tile([128, TC], f32, tag="ps_y", name="ps_y")
                    acc = acc_pool.tile([128, TC], f32, tag="acc", name="acc")
                    accb = accb_pool.tile([128, TC], dt, tag="accb", name="accb")
                    for idx, j in enumerate(order):
                        r = j - 4 * g
                        n0 = 128 * r if r > 0 else 0
                        ps_s = s_psum.tile([128, TC], f32, tag="s", name="ps_s")
                        nc.tensor.matmul(
                            ps_s[:, n0:TC],
                            kT[h][:, j * 128 : (j + 1) * 128],
                            qT[h][:, g * TC + n0 : (g + 1) * TC],
                            start=True,
                            stop=True,
                        )
                        pT = p_pool.tile([128, TC], dt, tag="p", name="pT")
                        nc.scalar.activation(
                            pT[:, n0:TC],
                            ps_s[:, n0:TC],
                            mybir.ActivationFunctionType.Exp,
                            scale=float(ATTN_MULT),
                        )
                        if r >= 0:
                            # causal mask on the diagonal 128-col sub-block:
                            # keep iff f' - p >= 0
                            nc.gpsimd.affine_select(
                                out=pT[:, n0 : n0 + 128],
                                in_=pT[:, n0 : n0 + 128],
                                compare_op=mybir.AluOpType.is_ge,
                                fill=0.0,
                                base=0,
                                pattern=[[1, 128]],
                                channel_multiplier=-1,
                            )
                        nc.tensor.matmul(
                            ps_y[:, n0:TC],
                            v_sb[j][:, h * 128 : (h + 1) * 128],
                            pT[:, n0:TC],
                            start=(idx == 0),
                            stop=(idx == len(order) - 1),
                        )
                        if idx == 0:
                            nc.vector.tensor_copy(acc[:], pT[:])
                        elif idx == len(order) - 1 and g > 0:
                            # fused final accumulate + convert to bf16
                            nc.vector.tensor_add(accb[:], acc[:], pT[:])
                        else:
                            nc.vector.tensor_add(
                                acc[:, n0:TC], acc[:, n0:TC], pT[:, n0:TC]
                            )
                    if g == 0:
                        nc.vector.tensor_copy(accb[:], acc[:])
                    ps_r = r_psum.tile([128, TC], f32, tag="ps_r", name="ps_r")
                    nc.tensor.matmul(
                        ps_r[:], ones128[:], accb[:], start=True, stop=True
                    )
                    recip = recip_pool.tile(
                        [128, TC], f32, tag="recip", name="recip"
                    )
                    nc.vector.reciprocal_approx_fast(out=recip[:], in_=ps_r[:])
                    y_sb = y_pool.tile([128, TC], dt, tag="y", name="y_sb")
                    nc.vector.tensor_mul(y_sb[:], ps_y[:], recip[:])
                    y_tiles[(g, h)] = y_sb

            def proj_group(g):
                for co in range(C // 128):
                    ps_o = chain_psum.tile([128, TC], f32, tag="mm", name="ps_o")
                    for ci in range(N_HEAD_CORE):
                        nc.tensor.matmul(
                            ps_o[:],
                            wp_t[ci][:, co * 128 : (co + 1) * 128],
                            y_tiles[(g, ci)][:],
                            start=(ci == 0),
                            stop=(ci == N_HEAD_CORE - 1),
                        )
                    o_sb = o_pool.tile([128, TC], dt, tag="o", name="o_sb")
                    if co % 2 == 0:
                        nc.scalar.copy(o_sb[:], ps_o[:])
                    else:
                        nc.vector.tensor_copy(o_sb[:], ps_o[:])
                    nc.sync.dma_start(
                        out=outT.ap()[
                            co * 128 : (co + 1) * 128, g * TC : (g + 1) * TC
                        ],
                        in_=o_sb[:],
                    )

            # emission order = scheduler priority
            for g in range(n_rounds):
                qk_round(g)
                v_round(g)
                attn_group(g)
                if g == 3:
                    proj_group(0)
            proj_group(1)
            proj_group(2)
            proj_group(3)

    nc.compile()
    return nc


def kernel(x, w_qkv, b_qkv, w_proj, b_proj, _trace=False):
    x = np.ascontiguousarray(np.asarray(x, dtype=np.float32))
    w_qkv = np.ascontiguousarray(np.asarray(w_qkv, dtype=np.float32))
    b_qkv = np.ascontiguousarray(np.asarray(b_qkv, dtype=np.float32))
    w_proj = np.ascontiguousarray(np.asarray(w_proj, dtype=np.float32))
    b_proj = np.ascontiguousarray(np.asarray(b_proj, dtype=np.float32))
    B = x.shape[0]

    if "nc" not in _CACHED:
        _CACHED["nc"] = build_nc()
    nc = _CACHED["nc"]

    np_dt = ml_dtypes.bfloat16

    def cvt(a):
        return np.ascontiguousarray(a.astype(np_dt))

    in_maps = []
    for core in range(N_CORES):
        b, hg = divmod(core, 4)
        s = slice(hg * JW, (hg + 1) * JW)
        in_maps.append(
            {
                "xt": cvt(np.ascontiguousarray(x[b].T)),
                "wq": cvt(w_qkv[:, 0:C][:, s]),
                "wk": cvt(w_qkv[:, C : 2 * C][:, s]),
                "wv": cvt(w_qkv[:, 2 * C : 3 * C][:, s]),
                "wp": cvt(w_proj[s, :]),
                "bq": np.ascontiguousarray(b_qkv[0:C][s]),
                "bk": np.ascontiguousarray(b_qkv[C : 2 * C][s]),
                "bv": cvt(b_qkv[2 * C : 3 * C][s]),
                "ones": np.ones((128, 128), dtype=np_dt),
            }
        )

    res = run_bass_kernel_spmd(nc, in_maps, list(range(N_CORES)), trace=_trace)
    _CACHED["last_result"] = res

    out = np.zeros((B, T, C), dtype=np.float32)
    for core in range(N_CORES):
        b, hg = divmod(core, 4)
        out[b] += res.results[core]["outT"].T.astype(np.float32)
    out += b_proj
    return np.ascontiguousarray(out)
